# revision 3
# baseline (speedup 1.0000x reference)
"""EEGFormer transformer-block kernel for 8 Trainium2 NeuronCores.

Strategy: pure data parallelism. The B*S = 128 attention slices are
independent; each of the 8 cores processes 16 slices ([256 tokens, 512
features] each) end-to-end with a fully replicated weight set. No
collectives.

Per-core kernel (Bass/Tile): processes 8 "megatiles" of 512 tokens
(2 slices). Matmuls run in bf16 (PE 1 cyc/row); statistics, softmax
accumulation, and residuals stay fp32.
"""

import os
import sys

import numpy as np

if "/opt/trn_rl_repo" not in sys.path and os.path.isdir("/opt/trn_rl_repo"):
    sys.path.insert(0, "/opt/trn_rl_repo")

B, S, C, L = 4, 32, 256, 512
H = 8
D = L // H
FL = 4 * L  # FFN hidden 2048
EPS = 1e-5
N_CORES = 8
SLICES = (B * S) // N_CORES       # 16 slices per core
MT_SLICES = 2                      # slices per megatile
N_MT = SLICES // MT_SLICES         # 8 megatiles
TOK = C * MT_SLICES                # 512 tokens per megatile
TC = TOK // 128                    # 4 token chunks
LC = L // 128                      # 4 feature chunks
FC = FL // 128                     # 16 ffn-hidden chunks

_cache = {}


def _build(mm_bf16=True):
    import concourse.bacc as bacc
    import concourse.mybir as mybir
    import concourse.tile as tile
    from concourse.masks import make_identity

    f32 = mybir.dt.float32
    mdt = mybir.dt.bfloat16 if mm_bf16 else mybir.dt.float32
    AF = mybir.ActivationFunctionType
    OP = mybir.AluOpType

    nc = bacc.Bacc("TRN2", target_bir_lowering=False)

    x_d = nc.dram_tensor("x", [SLICES, C, L], f32, kind="ExternalInput")
    wq_d = nc.dram_tensor("wqT", [L, L], mdt, kind="ExternalInput")
    wk_d = nc.dram_tensor("wkT", [L, L], mdt, kind="ExternalInput")
    wv_d = nc.dram_tensor("wvT", [L, L], mdt, kind="ExternalInput")
    wo_d = nc.dram_tensor("woT", [L, L], mdt, kind="ExternalInput")
    w1_d = nc.dram_tensor("w1T", [L, FL], mdt, kind="ExternalInput")
    w2_d = nc.dram_tensor("w2T", [FL, L], mdt, kind="ExternalInput")
    bo_d = nc.dram_tensor("bo", [L], f32, kind="ExternalInput")
    b1_d = nc.dram_tensor("b1", [FL], f32, kind="ExternalInput")
    b2_d = nc.dram_tensor("b2", [L], f32, kind="ExternalInput")
    g1_d = nc.dram_tensor("g1", [L], f32, kind="ExternalInput")
    be1_d = nc.dram_tensor("be1", [L], f32, kind="ExternalInput")
    g2_d = nc.dram_tensor("g2", [L], f32, kind="ExternalInput")
    be2_d = nc.dram_tensor("be2", [L], f32, kind="ExternalInput")
    out_d = nc.dram_tensor("out", [SLICES, C, L], f32, kind="ExternalOutput")

    # DRAM views: tokens grouped as [32 chunks of 128, 128, L]
    x_v = x_d[:, :, :].rearrange("s (tc p) l -> (s tc) p l", p=128)
    out_v = out_d[:, :, :].rearrange("s (tc p) l -> (s tc) p l", p=128)

    import concourse.bass as bass

    def bcast_row(vec_ap, p=128):
        # DMA-broadcast a [n] DRAM vector across p partitions -> [p, n]
        return bass.AP(
            tensor=vec_ap.tensor,
            offset=vec_ap.offset,
            ap=[[0, p]] + list(vec_ap.ap),
        )

    with tile.TileContext(nc) as tc_ctx:
        tc = tc_ctx
        import contextlib

        ctx = contextlib.ExitStack()
        with ctx:
            wpool = ctx.enter_context(tc.tile_pool(name="weights", bufs=1))
            const = ctx.enter_context(tc.tile_pool(name="const", bufs=1))
            xin = ctx.enter_context(tc.tile_pool(name="xin", bufs=2))
            act = ctx.enter_context(tc.tile_pool(name="act", bufs=1))
            sm = ctx.enter_context(tc.tile_pool(name="sm", bufs=4))
            yp = ctx.enter_context(tc.tile_pool(name="yp", bufs=3))
            outp = ctx.enter_context(tc.tile_pool(name="outp", bufs=2))
            stat = ctx.enter_context(tc.tile_pool(name="stat", bufs=4))
            ps_acc = ctx.enter_context(tc.tile_pool(name="ps_acc", bufs=4, space="PSUM"))
            ps_big = ctx.enter_context(tc.tile_pool(name="ps_big", bufs=2, space="PSUM"))
            ps_att = ctx.enter_context(tc.tile_pool(name="ps_att", bufs=2, space="PSUM"))

            # ---- constants / weights (loaded once) ----
            wq_s = wpool.tile([128, LC, L], mdt)
            wk_s = wpool.tile([128, LC, L], mdt)
            wv_s = wpool.tile([128, LC, L], mdt)
            wo_s = wpool.tile([128, LC, L], mdt)
            w1_s = wpool.tile([128, LC, FL], mdt)
            w2_s = wpool.tile([128, FC, L], mdt)
            for dst, src in ((wq_s, wq_d), (wk_s, wk_d), (wv_s, wv_d), (wo_s, wo_d), (w1_s, w1_d)):
                nc.sync.dma_start(out=dst, in_=src[:, :].rearrange("(kc p) f -> p kc f", p=128))
            nc.sync.dma_start(out=w2_s, in_=w2_d[:, :].rearrange("(kc p) f -> p kc f", p=128))

            ident = const.tile([128, 128], mdt)
            make_identity(nc, ident)
            eps_t = const.tile([128, 1], f32)
            nc.vector.memset(eps_t, EPS)
            g1_s = const.tile([128, LC], f32)
            be1_s = const.tile([128, LC], f32)
            g2_s = const.tile([128, LC], f32)
            be2_s = const.tile([128, LC], f32)
            b1_s = const.tile([128, FC], f32)
            for dst, src in ((g1_s, g1_d), (be1_s, be1_d), (g2_s, g2_d), (be2_s, be2_d), (b1_s, b1_d)):
                nc.sync.dma_start(out=dst, in_=src[:].rearrange("(c p) -> p c", p=128))
            bo_b = const.tile([128, L], f32)
            b2_b = const.tile([128, L], f32)
            nc.gpsimd.dma_start(out=bo_b, in_=bcast_row(bo_d[:]))
            nc.gpsimd.dma_start(out=b2_b, in_=bcast_row(b2_d[:]))

            def layernorm_T(x_sb, g_s, be_s, name, mt):
                """LN over feature dim of x_sb [128, TC, L] (fp32, tokens on
                partitions), producing the normalized TRANSPOSE hT [128, LC, TOK]
                (mdt, features on partitions) via PE transposes."""
                xcn = act.tile([128, TC, L], mdt, name=f"xcn_{name}", tag=f"xcn_{name}")
                mv = stat.tile([128, TC, 2], f32, name=f"mv_{name}", tag="mv")
                rstd = stat.tile([128, TC], f32, name=f"rstd_{name}", tag="rstd")
                bn = stat.tile([128, 6], f32, name=f"bn_{name}", tag="bn")
                for t in range(TC):
                    nc.vector.bn_stats(out=bn, in_=x_sb[:, t, :])
                    nc.vector.bn_aggr(out=mv[:, t, :], in_=bn)
                    nc.scalar.activation(
                        out=rstd[:, t : t + 1], in_=mv[:, t, 1:2],
                        func=AF.Sqrt, bias=eps_t, scale=1.0,
                    )
                    nc.vector.reciprocal(out=rstd[:, t : t + 1], in_=rstd[:, t : t + 1])
                    nc.vector.tensor_scalar(
                        out=xcn[:, t, :], in0=x_sb[:, t, :],
                        scalar1=mv[:, t, 0:1], scalar2=rstd[:, t : t + 1],
                        op0=OP.subtract, op1=OP.mult,
                    )
                hT = act.tile([128, LC, TOK], mdt, name=f"hT_{name}", tag=f"hT_{name}")
                for m in range(LC):
                    hps = ps_acc.tile([128, TOK], f32, name=f"hps_{name}_{mt}_{m}", tag="ps_acc")
                    for t in range(TC):
                        nc.tensor.matmul(
                            hps[:, t * 128 : (t + 1) * 128],
                            xcn[:, t, m * 128 : (m + 1) * 128],
                            ident,
                        )
                    nc.vector.tensor_scalar(
                        out=hT[:, m, :], in0=hps,
                        scalar1=g_s[:, m : m + 1], scalar2=be_s[:, m : m + 1],
                        op0=OP.mult, op1=OP.add,
                    )
                return hT

            for mt in range(N_MT):
                # ---- load x megatile ----
                x_sb = xin.tile([128, TC, L], f32, name=f"x_{mt}", tag="x")
                nc.sync.dma_start(
                    out=x_sb,
                    in_=x_v[4 * mt : 4 * mt + 4].rearrange("c p l -> p c l"),
                )

                # ---- LN1 (transposed, normalized) ----
                hT = layernorm_T(x_sb, g1_s, be1_s, "ln1", mt)

                # ---- QKV projections ----
                # qT/kT: [feat 128, LC, TOK]; v: [tok 128, TC, L(dfeat)]
                qT = act.tile([128, LC, TOK], mdt, name=f"qT_{mt}", tag="qT")
                kT = act.tile([128, LC, TOK], mdt, name=f"kT_{mt}", tag="kT")
                v_sb = act.tile([128, TC, L], mdt, name=f"v_{mt}", tag="v")
                for m in range(LC):
                    for dst, w_s, eng in ((qT, wq_s, nc.vector), (kT, wk_s, nc.scalar)):
                        pq = ps_big.tile([128, TOK], f32, name=f"psqk_{mt}_{m}", tag="ps_big")
                        for kc in range(LC):
                            nc.tensor.matmul(
                                pq, w_s[:, kc, m * 128 : (m + 1) * 128], hT[:, kc, :],
                                start=(kc == 0), stop=(kc == LC - 1),
                            )
                        if eng is nc.scalar:
                            nc.scalar.copy(out=dst[:, m, :], in_=pq)
                        else:
                            nc.vector.tensor_copy(out=dst[:, m, :], in_=pq)
                for t in range(TC):
                    pv = ps_big.tile([128, L], f32, name=f"psv_{mt}_{t}", tag="ps_big")
                    for kc in range(LC):
                        nc.tensor.matmul(
                            pv, hT[:, kc, t * 128 : (t + 1) * 128], wv_s[:, kc, :],
                            start=(kc == 0), stop=(kc == LC - 1),
                        )
                    nc.vector.tensor_copy(out=v_sb[:, t, :], in_=pv)

                # ---- attention (per slice, per head) ----
                # oT accumulators: [dfeat 128, TOK] per feature chunk
                oT_ps = [
                    ps_acc.tile([128, TOK], f32, name=f"oT_{mt}_{m}", tag="ps_acc")
                    for m in range(LC)
                ]
                for sl in range(MT_SLICES):
                    t0 = sl * (C // 128)  # first token chunk of this slice
                    tok_sl = slice(sl * C, (sl + 1) * C)
                    for h in range(H):
                        m = h // 2
                        prow = (h % 2) * 64
                        q_h = qT[prow : prow + 64, m, tok_sl]
                        k_h = kT[prow : prow + 64, m, tok_sl]
                        pexp = sm.tile([128, 2, C], mdt, name=f"pexp_{mt}_{sl}_{h}", tag="pexp")
                        zz = stat.tile([128, 2], f32, name=f"z_{mt}_{sl}_{h}", tag="z")
                        rz = stat.tile([128, 2], f32, name=f"rz_{mt}_{sl}_{h}", tag="rz")
                        for qc in range(2):
                            sps = ps_att.tile([128, C], f32, name=f"s_{mt}_{sl}_{h}_{qc}", tag="ps_att")
                            nc.tensor.matmul(
                                sps, q_h[:, qc * 128 : (qc + 1) * 128], k_h,
                            )
                            nc.scalar.activation(
                                out=pexp[:, qc, :], in_=sps, func=AF.Exp,
                                scale=float(D) ** -0.5,
                                accum_out=zz[:, qc : qc + 1],
                            )
                        nc.vector.reciprocal(out=rz, in_=zz)
                        # P^T with 1/Z folded in via diag
                        pT_ps = [None, None]
                        for kc in range(2):
                            pT_ps[kc] = ps_att.tile(
                                [128, C], f32, name=f"pt_{mt}_{sl}_{h}_{kc}", tag="ps_att"
                            )
                        diag = sm.tile([128, 2, 128], mdt, name=f"diag_{mt}_{sl}_{h}", tag="diag")
                        for qc in range(2):
                            nc.vector.tensor_scalar_mul(
                                diag[:, qc, :], ident, rz[:, qc : qc + 1]
                            )
                            for kc in range(2):
                                nc.tensor.matmul(
                                    pT_ps[kc][:, qc * 128 : (qc + 1) * 128],
                                    pexp[:, qc, kc * 128 : (kc + 1) * 128],
                                    diag[:, qc, :],
                                )
                        pT = sm.tile([128, 2, C], mdt, name=f"pTs_{mt}_{sl}_{h}", tag="pTs")
                        for kc in range(2):
                            nc.vector.tensor_copy(out=pT[:, kc, :], in_=pT_ps[kc])
                            nc.tensor.matmul(
                                oT_ps[m][prow : prow + 64, tok_sl],
                                v_sb[:, t0 + kc, h * 64 : (h + 1) * 64],
                                pT[:, kc, :],
                                start=(kc == 0), stop=(kc == 1),
                            )

                oT = act.tile([128, LC, TOK], mdt, name=f"oTs_{mt}", tag="oTs")
                for m in range(LC):
                    nc.vector.tensor_copy(out=oT[:, m, :], in_=oT_ps[m])

                # ---- output projection + residual + bo ----
                xa = act.tile([128, TC, L], f32, name=f"xa_{mt}", tag="xa")
                for t in range(TC):
                    pxa = ps_acc.tile([128, L], f32, name=f"pxa_{mt}_{t}", tag="ps_acc")
                    for kc in range(LC):
                        nc.tensor.matmul(
                            pxa, oT[:, kc, t * 128 : (t + 1) * 128], wo_s[:, kc, :],
                            start=(kc == 0), stop=(kc == LC - 1),
                        )
                    nc.vector.tensor_add(out=xa[:, t, :], in0=pxa, in1=x_sb[:, t, :])
                    nc.vector.tensor_add(out=xa[:, t, :], in0=xa[:, t, :], in1=bo_b)

                # ---- LN2 ----
                h2T = layernorm_T(xa, g2_s, be2_s, "ln2", mt)

                # ---- FFN ----
                pf = [
                    ps_acc.tile([128, L], f32, name=f"pf_{mt}_{t}", tag="ps_acc")
                    for t in range(TC)
                ]
                for fc in range(FC):
                    py = ps_big.tile([128, TOK], f32, name=f"py_{mt}_{fc}", tag="ps_big")
                    for kc in range(LC):
                        nc.tensor.matmul(
                            py, w1_s[:, kc, fc * 128 : (fc + 1) * 128], h2T[:, kc, :],
                            start=(kc == 0), stop=(kc == LC - 1),
                        )
                    yT = yp.tile([128, TOK], mdt, name=f"yT_{mt}_{fc}", tag="yT")
                    nc.scalar.activation(
                        out=yT, in_=py, func=AF.Relu,
                        bias=b1_s[:, fc : fc + 1], scale=1.0,
                    )
                    for t in range(TC):
                        nc.tensor.matmul(
                            pf[t], yT[:, t * 128 : (t + 1) * 128], w2_s[:, fc, :],
                            start=(fc == 0), stop=(fc == FC - 1),
                        )

                o_sb = outp.tile([128, TC, L], f32, name=f"o_{mt}", tag="o")
                for t in range(TC):
                    nc.vector.tensor_add(out=o_sb[:, t, :], in0=pf[t], in1=xa[:, t, :])
                    nc.vector.tensor_add(out=o_sb[:, t, :], in0=o_sb[:, t, :], in1=b2_b)
                nc.sync.dma_start(
                    out=out_v[4 * mt : 4 * mt + 4].rearrange("c p l -> p c l"),
                    in_=o_sb,
                )

    nc.finalize()
    return nc


def _get_nc():
    mm_bf16 = os.environ.get("EEGK_FP32", "0") != "1"
    key = ("nc", mm_bf16)
    if key not in _cache:
        _cache[key] = _build(mm_bf16=mm_bf16)
    return _cache[key]


def _install_ntff_shim():
    """Provide antenv.axon_hooks so trace=True works under axon."""
    import types
    import contextlib as _cl

    if "antenv.axon_hooks" in sys.modules:
        return
    mod = types.ModuleType("antenv.axon_hooks")
    mod._hook = None
    mod.set_axon_ntff_profile_hook = lambda h: setattr(mod, "_hook", h)
    mod.get_axon_ntff_profile_hook = lambda: mod._hook
    sys.modules["antenv.axon_hooks"] = mod
    try:
        import antenv

        antenv.axon_hooks = mod
        from trn_agent_boot import trn_boot

        hook = trn_boot._ntff_profile_via_ctypes("/opt/axon/libaxon_pjrt.so")
        mod.set_axon_ntff_profile_hook(hook)
    except Exception:
        pass


last_exec_ns = None
last_results = None


def kernel(**inputs):
    global last_exec_ns, last_results
    from concourse.bass_utils import run_bass_kernel_spmd
    import ml_dtypes

    mm_bf16 = os.environ.get("EEGK_FP32", "0") != "1"
    mdt_np = ml_dtypes.bfloat16 if mm_bf16 else np.float32
    nc = _get_nc()

    x = np.asarray(inputs["x"], dtype=np.float32)
    Wq = np.asarray(inputs["Wq"], dtype=np.float32)
    Wk = np.asarray(inputs["Wk"], dtype=np.float32)
    Wv = np.asarray(inputs["Wv"], dtype=np.float32)
    Wo = np.asarray(inputs["Wo"], dtype=np.float32)

    def headT(w):  # [H, D, L] -> [L, H*D]
        return np.ascontiguousarray(w.transpose(2, 0, 1).reshape(L, L))

    shared = {
        "wqT": headT(Wq).astype(mdt_np),
        "wkT": headT(Wk).astype(mdt_np),
        "wvT": headT(Wv).astype(mdt_np),
        "woT": np.ascontiguousarray(Wo.T).astype(mdt_np),
        "w1T": np.ascontiguousarray(np.asarray(inputs["W1"], np.float32).T).astype(mdt_np),
        "w2T": np.ascontiguousarray(np.asarray(inputs["W2"], np.float32).T).astype(mdt_np),
        "bo": np.asarray(inputs["bo"], np.float32),
        "b1": np.asarray(inputs["b1"], np.float32),
        "b2": np.asarray(inputs["b2"], np.float32),
        "g1": np.asarray(inputs["g1"], np.float32),
        "be1": np.asarray(inputs["be1"], np.float32),
        "g2": np.asarray(inputs["g2"], np.float32),
        "be2": np.asarray(inputs["be2"], np.float32),
    }
    x_sl = np.ascontiguousarray(x.reshape(B * S, C, L))
    in_maps = [
        {"x": x_sl[i * SLICES : (i + 1) * SLICES], **shared} for i in range(N_CORES)
    ]

    trace = os.environ.get("EEGK_TRACE", "0") == "1"
    if trace:
        _install_ntff_shim()
    res = run_bass_kernel_spmd(nc, in_maps, core_ids=list(range(N_CORES)), trace=trace)
    last_exec_ns = res.exec_time_ns
    last_results = res
    out = np.concatenate([res.results[i]["out"] for i in range(N_CORES)], axis=0)
    return out.reshape(B, S, C, L).astype(np.float32)
